# revision 9
# baseline (speedup 1.0000x reference)
"""Sparse (sliding-window + sink) GQA attention on 8 NeuronCores, bf16.

Sharding: tensor-parallel over heads. Core c owns q-heads {2c, 2c+1} and
kv-head c//2. Each core computes its heads' attention and a partial
output projection (wo columns for its heads); host sums the 8 partials.

All matmuls in bf16 (1 cyc/row on PE regardless of free size), fp32 PSUM
accumulation. Attention runs in transposed orientation ST[k, q]; softmax
denominator via ones-vector matmul; normalization folded into the
PSUM->SBUF eviction of the output. RMS-norm is folded into RoPE via
scalar_tensor_tensor; the sliding-window masks are binary bf16 multiplies
applied post-exp (scores*SCALE are bounded by sqrt(D) so no overflow).
DMAs are batched (few large transfers) to minimize HWDGE serialization.
The kernel returns out^T in bf16; the host sums partials and transposes.
"""

import numpy as np
import ml_dtypes
from contextlib import ExitStack

import concourse.bass as bass
import concourse.bacc as bacc
import concourse.mybir as mybir
import concourse.tile as tile
from concourse.bass_utils import run_bass_kernel_spmd
from concourse.alu_op_type import AluOpType

S = 2048
H = 16
KVH = 4
D = 128
HID = H * D
WIN = 1024
EPS = 1e-5
NCORES = 8
F32 = mybir.dt.float32
BF16 = mybir.dt.bfloat16
AF = mybir.ActivationFunctionType
OP = AluOpType
BF = ml_dtypes.bfloat16
SCALE = 1.0 / float(np.sqrt(D))
NSC = S // 128  # 16 s-chunks


def _build_kernel():
    nc = bacc.Bacc("TRN2", target_bir_lowering=False, debug=False)

    xTb = nc.dram_tensor("xTb", [HID, S], BF16, kind="ExternalInput").ap()
    wqkvT = nc.dram_tensor("wqkvT", [HID, 512], BF16, kind="ExternalInput").ap()
    woTd = nc.dram_tensor("woTd", [256, HID], BF16, kind="ExternalInput").ap()
    cosd = nc.dram_tensor("cosd", [S, 384], BF16, kind="ExternalInput").ap()
    sind = nc.dram_tensor("sind", [S, 384], BF16, kind="ExternalInput").ap()
    maskd = nc.dram_tensor("maskd", [128, 256], BF16, kind="ExternalInput").ap()
    identd = nc.dram_tensor("identd", [128, 128], BF16, kind="ExternalInput").ap()
    esinkd = nc.dram_tensor("esinkd", [1, 2], F32, kind="ExternalInput").ap()
    outT = nc.dram_tensor("outT", [HID, S], BF16, kind="ExternalOutput").ap()

    with tile.TileContext(nc) as tc:
        with ExitStack() as ctx:
            _emit(ctx, tc, nc, xTb, wqkvT, woTd, cosd, sind, maskd, identd,
                  esinkd, outT)
    nc.compile()
    return nc


def _emit(ctx, tc, nc, xTb, wqkvT, woTd, cosd, sind, maskd, identd, esinkd,
          outT):
    pers = ctx.enter_context(tc.tile_pool(name="pers", bufs=1))
    xtp = ctx.enter_context(tc.tile_pool(name="xtp", bufs=12))
    qkrp = ctx.enter_context(tc.tile_pool(name="qkrp", bufs=5))
    smp = ctx.enter_context(tc.tile_pool(name="smp", bufs=4))
    esp = ctx.enter_context(tc.tile_pool(name="esp", bufs=24))
    opp = ctx.enter_context(tc.tile_pool(name="opp", bufs=3))
    psM = ctx.enter_context(tc.tile_pool(name="psM", bufs=3, space="PSUM"))
    psOD = ctx.enter_context(tc.tile_pool(name="psOD", bufs=2, space="PSUM"))
    psS = ctx.enter_context(tc.tile_pool(name="psS", bufs=3, space="PSUM"))

    # persistent SBUF
    wqkvb = pers.tile([128, 16 * 512], BF16, tag="wqkvb")
    cosb = pers.tile([128, 16 * 384], BF16, tag="cosb")
    sinb = pers.tile([128, 16 * 384], BF16, tag="sinb")
    QT = [pers.tile([128, S], BF16, tag=f"QT{h}", name=f"QT{h}")
          for h in range(2)]
    KT = pers.tile([128, S], BF16, tag="KT")
    qkv_all = pers.tile([128, 16 * 512], BF16, tag="qkv_all")
    attnT = [pers.tile([128, S], BF16, tag=f"attnT{h}", name=f"attnT{h}")
             for h in range(2)]
    woTs = pers.tile([128, 2 * 2048], BF16, tag="woTs")
    out_t = pers.tile([128, 16 * 512], BF16, tag="out_t")
    mskb = pers.tile([128, 256], BF16, tag="mskb")
    idt = pers.tile([128, 128], BF16, tag="idt")
    esink = pers.tile([1, 2], F32, tag="esink")
    ones = pers.tile([128, 1], BF16, tag="ones")
    junk = pers.tile([128, 384], BF16, tag="junk")

    nc.vector.memset(junk[:], 0.0)
    nc.vector.memset(ones[:], 1.0)

    wb3 = wqkvb[:].rearrange("p (n q) -> p n q", n=16)
    wq3 = wqkvT.rearrange("(n p) q -> p n q", p=128)
    x3 = xTb.rearrange("(n p) s -> p n s", p=128)
    cs3 = cosb[:].rearrange("p (n d) -> p n d", n=16)
    cd3 = cosd.rearrange("(n p) d -> p n d", p=128)
    ss3_ = sinb[:].rearrange("p (n d) -> p n d", n=16)
    sd3 = sind.rearrange("(n p) d -> p n d", p=128)
    wo3 = woTs[:].rearrange("p (i e) -> p i e", i=2)
    wod3 = woTd.rearrange("(i p) e -> p i e", p=128)
    qv3 = qkv_all[:].rearrange("p (n q) -> p n q", n=16)
    ot3 = out_t[:].rearrange("p (n s) -> p n s", n=16)
    oT3 = outT.rearrange("(n p) s -> p n s", p=128)

    # ---- x tiles: one per (half-sb, ec-quarter): [128, 4 ec, 256 s] ----
    xtiles = {}

    def emit_xload(hsb, ecq):
        xt = xtp.tile([128, 4 * 256], BF16, tag="xt", name=f"xt{hsb}_{ecq}")
        xta = xt[:].rearrange("p (n s) -> p n s", n=4)
        nc.sync.dma_start(
            xta[:], x3[:, ecq * 4:(ecq + 1) * 4, hsb * 256:(hsb + 1) * 256])
        xtiles[(hsb, ecq)] = xta
        return xta

    # startup: identity first (feeds PE warmup), then wqkv quarters
    # interleaved with x half-sb-0 quarters so the first accumulation
    # chain can start as soon as possible; everything else ordered by
    # first-use time (HWDGE + DMA engines are serial)
    for q4 in range(4):
        nc.sync.dma_start(wb3[:, q4 * 4:(q4 + 1) * 4, :],
                          wq3[:, q4 * 4:(q4 + 1) * 4, :])
        emit_xload(0, q4)
    nc.sync.dma_start(idt[:], identd[:])
    # PE warmup: dummy matmuls (colsums of a memset tile) ramp the tensor
    # engine to full p-state while the first x/weight DMAs are in flight
    for wu in range(46):
        wps = psS.tile([128, 512], F32, tag="s", name=f"wups{wu}")
        nc.tensor.matmul(wps[0:1, 0:128], ones[:], junk[:, 0:128],
                         start=True, stop=True)
    for q4 in range(4):
        emit_xload(1, q4)
    nc.sync.dma_start(cs3[:, 0:8, :], cd3[:, 0:8, :])
    nc.sync.dma_start(ss3_[:, 0:8, :], sd3[:, 0:8, :])
    for q4 in range(4):
        emit_xload(2, q4)
    nc.sync.dma_start(cs3[:, 8:16, :], cd3[:, 8:16, :])
    nc.sync.dma_start(ss3_[:, 8:16, :], sd3[:, 8:16, :])
    for q4 in range(4):
        emit_xload(3, q4)
    nc.sync.dma_start(wo3[:], wod3[:])
    nc.sync.dma_start(mskb[:], maskd[:])
    nc.sync.dma_start(esink[:], esinkd[:])
    esinkb = pers.tile([128, 2], F32, tag="esinkb")
    nc.gpsimd.partition_broadcast(esinkb[:], esink[:])

    # ---- Phase A: proj + rms + rope + transpose ----
    wu_n = 46

    def emit_dummies(n):
        # keep the PE continuously busy (p-state stays ramped) while the
        # startup DMA feed catches up
        nonlocal wu_n
        for _ in range(n):
            wps = psS.tile([128, 512], F32, tag="s", name=f"wups{wu_n}")
            nc.tensor.matmul(wps[0:1, 0:128], ones[:], junk[:, 0:128],
                             start=True, stop=True)
            wu_n += 1

    def emit_proj(sc):
        hsb, off = sc // 2, (sc % 2) * 128
        qkvp = psM.tile([128, 512], F32, tag="m", name=f"qkvp{sc}")
        for ec in range(16):
            xta = xtiles[(hsb, ec // 4)]
            nc.tensor.matmul(qkvp[:], xta[:, ec % 4, off:off + 128],
                             wb3[:, ec, :], start=(ec == 0), stop=(ec == 15))
        # rms stats: Square+accum on Act (square lives in the exp act-func
        # set, so Act stays on one table). rsqrt on DVE via reciprocal
        # seed + one Newton step: ms concentrates near 1, rel err <~0.1%.
        # eps=1e-5 is negligible against ms~1 and is dropped.
        ssq = smp.tile([128, 3], F32, tag="ss", name=f"ssq{sc}")
        for hh in range(3):
            nc.scalar.activation(junk[:, hh * 128:(hh + 1) * 128],
                                 qkvp[:, hh * 128:(hh + 1) * 128],
                                 AF.Square, scale=float(D) ** -0.5,
                                 accum_out=ssq[:, hh:hh + 1])
        rr = smp.tile([128, 3], F32, tag="rr", name=f"rr{sc}")
        nc.vector.reciprocal_approx_fast(rr[:], ssq[:])
        y0 = smp.tile([128, 3], F32, tag="y0", name=f"y0{sc}")
        nc.vector.tensor_scalar(out=y0[:], in0=rr[:], scalar1=1.0,
                                scalar2=0.5, op0=OP.add, op1=OP.mult)
        tn = smp.tile([128, 3], F32, tag="tn", name=f"tn{sc}")
        nc.vector.tensor_mul(tn[:], y0[:], y0[:])
        nc.vector.tensor_mul(tn[:], tn[:], ssq[:])
        nc.vector.tensor_scalar(out=tn[:], in0=tn[:], scalar1=-0.5,
                                scalar2=1.5, op0=OP.mult, op1=OP.add)
        iv = smp.tile([128, 3], F32, tag="iv", name=f"iv{sc}")
        nc.vector.tensor_mul(iv[:], y0[:], tn[:])
        # evict whole qkv chunk to bf16 SBUF (V stays here for the PV matmul)
        nc.scalar.copy(qkv_all[:, sc * 512:(sc + 1) * 512], qkvp[:])
        # rope with rms-scale folded in:
        #   qkr = (q*iv)*cosw + (rot(q)*iv)*sinw
        # cos side on Pool, rot side on DVE to balance engine load
        qs = qkv_all[:, sc * 512:sc * 512 + 384]
        qkr = qkrp.tile([128, 384], BF16, tag="qkr", name=f"qkr{sc}")
        rot = qkrp.tile([128, 384], BF16, tag="rot", name=f"rot{sc}")
        for hh in range(3):
            o = hh * 128
            nc.vector.scalar_tensor_tensor(
                qkr[:, o:o + 128], qs[:, o:o + 128], iv[:, hh:hh + 1],
                cs3[:, sc, o:o + 128], OP.mult, OP.mult)
            nc.vector.scalar_tensor_tensor(
                rot[:, o:o + 64], qs[:, o + 64:o + 128], iv[:, hh:hh + 1],
                ss3_[:, sc, o:o + 64], OP.mult, OP.mult)
            nc.vector.scalar_tensor_tensor(
                rot[:, o + 64:o + 128], qs[:, o:o + 64], iv[:, hh:hh + 1],
                ss3_[:, sc, o + 64:o + 128], OP.mult, OP.mult)
        nc.vector.tensor_add(qkr[:], qkr[:], rot[:])
        return qkr

    def emit_qtrans(sc, qkr):
        tp = psM.tile([128, 384], BF16, tag="m", name=f"tp{sc}")
        dests = [QT[0], QT[1], KT]
        for hh in range(3):
            nc.tensor.transpose(tp[:, hh * 128:(hh + 1) * 128],
                                qkr[:, hh * 128:(hh + 1) * 128], idt[:])
        for hh in range(3):
            dst = dests[hh][:, sc * 128:(sc + 1) * 128]
            if hh == 1:
                nc.scalar.copy(dst, tp[:, hh * 128:(hh + 1) * 128])
            else:
                nc.vector.tensor_copy(dst, tp[:, hh * 128:(hh + 1) * 128])

    # ---- Phase B producers (defined early so the proj tail can emit the
    # first attention scores+exps and hide the phase transition) ----
    groups = []
    for t in range(4):
        for h in range(2):
            groups.append((h, t, list(range(max(0, 4 * t - 8), 4 * (t + 1)))))

    all_work = []
    for h, t, kcs in groups:
        for kc in kcs:
            all_work.append((h, t, kc))

    pending = {}
    LOOKAHEAD = 20
    wi = 0

    def produce(h, t, kc):
        # score matmul -> exp (+ window mask as a binary multiply on Pool)
        jlo = max(0, kc - 4 * t)
        jhi = min(3, kc + 8 - 4 * t)
        a, b = jlo * 128, (jhi + 1) * 128
        qa, qb = t * 512 + a, t * 512 + b
        sp = psS.tile([128, 512], F32, tag="s", name=f"sp{h}_{t}_{kc}")
        nc.tensor.matmul(sp[:, a:b], KT[:, kc * 128:(kc + 1) * 128],
                         QT[h][:, qa:qb], start=True, stop=True)
        es = esp.tile([128, 512], BF16, tag="es", name=f"es{h}_{t}_{kc}")
        nc.scalar.activation(es[:, a:b], sp[:, a:b], AF.Exp, scale=SCALE)
        j = kc - 4 * t
        if 0 <= j < 4:
            nc.gpsimd.tensor_mul(es[:, j * 128:(j + 1) * 128],
                                 es[:, j * 128:(j + 1) * 128],
                                 mskb[:, 0:128])
        j2 = kc + 8 - 4 * t
        if 0 <= j2 < 4:
            nc.gpsimd.tensor_mul(es[:, j2 * 128:(j2 + 1) * 128],
                                 es[:, j2 * 128:(j2 + 1) * 128],
                                 mskb[:, 128:256])
        return es, a, b

    def prefetch(upto):
        nonlocal wi
        while wi < len(all_work) and wi < upto:
            hh, tt, kk = all_work[wi]
            pending[(hh, tt, kk)] = produce(hh, tt, kk)
            wi += 1

    # ---- proj loop; transposes run two s-chunks behind so the PE never
    # waits on the serial DVE stats/rope chain; the first attention
    # score+exp productions are interleaved into the tail ----
    qkr_hist = []
    for sc in range(NSC):
        if sc % 2 == 0 and sc // 2 + 4 < 8:
            for q4 in range(4):
                emit_xload(sc // 2 + 4, q4)
        qkr_hist.append(emit_proj(sc))
        if sc >= 3:
            emit_qtrans(sc - 3, qkr_hist[sc - 3])
        if sc >= 9:
            prefetch(4 * (sc - 8))
    for sc in (NSC - 3, NSC - 2, NSC - 1):
        emit_qtrans(sc, qkr_hist[sc])

    def emit_wo(t, jc0=0, jc1=16, split_evict=False):
        for jc in range(jc0, jc1):
            po = psM.tile([128, 512], F32, tag="m", name=f"po{jc}_{t}")
            for ic in range(2):
                nc.tensor.matmul(po[:], wo3[:, ic, jc * 128:(jc + 1) * 128],
                                 attnT[ic][:, t * 512:(t + 1) * 512],
                                 start=(ic == 0), stop=(ic == 1))
            dst = ot3[:, jc, :]
            if split_evict:
                nc.scalar.copy(dst[:, 0:256], po[:, 0:256])
                nc.vector.tensor_copy(dst[:, 256:512], po[:, 256:512])
            elif jc % 4 == (t % 4):
                nc.scalar.copy(dst, po[:])
            else:
                nc.vector.tensor_copy(dst, po[:])
            lo = None
            if t == 3:
                if jc in (1, 3):
                    lo = jc - 1
                elif jc in (7, 11):
                    lo = jc - 3
                elif jc == 14:
                    lo = 12
                elif jc == 15:
                    lo = 15
            elif jc % 4 == 3:
                lo = jc - 3
            if lo is not None:
                nc.sync.dma_start(
                    oT3[:, lo:jc + 1, t * 512:(t + 1) * 512],
                    ot3[:, lo:jc + 1, :])

    idx = 0
    for gi, (h, t, kcs) in enumerate(groups):
        op = psOD.tile([128, 512], F32, tag="od", name=f"op{h}_{t}")
        dn4 = psOD.tile([128, 4], F32, tag="od", name=f"dn4{h}_{t}")
        ncs = 0
        ncs_total = sum(
            min(3, k + 8 - 4 * t) - max(0, k - 4 * t) + 1 for k in kcs)
        for i, kc in enumerate(kcs):
            # wo for the previous q-block goes out mid-group, once its
            # attnT halves are long since normalized, so the PE never
            # waits on the normalize chain
            if gi in (3, 5) and i == 2:
                emit_wo(t - 1)
            if gi == 6 and i == 2:
                emit_wo(2, 0, 6)
            if gi == 7 and i == 2:
                emit_wo(2, 6, 12)
            prefetch(idx + 1 + LOOKAHEAD)
            es, a, b = pending.pop((h, t, kc))
            idx += 1
            first, last = (i == 0), (i == len(kcs) - 1)
            # denominator: es as STATIONARY, ones as 1-column moving ->
            # colsums land [128,1] (q on partitions) at ~1 streamed col.
            # PSUM start=True zeroes the WHOLE bank (probe-verified), so
            # only the group's very first colsum may carry start=True;
            # disjoint columns then accumulate per-element.
            for j in range(a // 128, b // 128):
                ncs += 1
                nc.tensor.matmul(
                    dn4[:, j:j + 1],
                    es[:, j * 128:(j + 1) * 128], ones[:],
                    start=(ncs == 1), stop=(ncs == ncs_total),
                    skip_group_check=True)
            nc.tensor.matmul(op[:, a:b], qv3[:, kc, 384:512], es[:, a:b],
                             start=first, stop=last)
        # evict op to SBUF right away so the PSUM slot frees without
        # waiting on the recip/broadcast chain; normalize from SBUF later
        ope = opp.tile([128, 512], BF16, tag="ope", name=f"ope{h}_{t}")
        nc.vector.tensor_copy(ope[:], op[:])
        dn4s = smp.tile([128, 4], F32, tag="dn", name=f"dn{h}_{t}")
        nc.vector.tensor_scalar_add(dn4s[:], dn4[:], esinkb[:, h:h + 1])
        nc.vector.reciprocal(dn4s[:], dn4s[:])
        dnr = smp.tile([1, 512], F32, tag="dnr", name=f"dnr{h}_{t}")
        for j in range(4):
            nc.sync.dma_start(dnr[:, j * 128:(j + 1) * 128],
                              dn4s[:, j:j + 1])
        db = smp.tile([128, 512], F32, tag="db", name=f"db{h}_{t}")
        nc.gpsimd.partition_broadcast(db[:], dnr[:])
        nc.vector.tensor_mul(attnT[h][:, t * 512:(t + 1) * 512], ope[:], db[:])
        if gi == len(groups) - 1:
            emit_wo(2, 12, 16, split_evict=True)
            emit_wo(3, split_evict=True)


_NC_CACHE = {}


def _get_nc():
    if "nc" not in _NC_CACHE:
        _NC_CACHE["nc"] = _build_kernel()
    return _NC_CACHE["nc"]


def kernel(x, cos, sin, wq, wk, wv, wo, sinks, q_norm_w, k_norm_w):
    x = np.asarray(x, np.float32).reshape(S, HID)
    xTh = np.ascontiguousarray(x.T).astype(BF)
    cos = np.asarray(cos, np.float32)
    sin = np.asarray(sin, np.float32)
    wq = np.asarray(wq, np.float32)
    wk = np.asarray(wk, np.float32)
    wv = np.asarray(wv, np.float32)
    wo = np.asarray(wo, np.float32)
    sinks = np.asarray(sinks, np.float32)
    qw = np.asarray(q_norm_w, np.float32)
    kw = np.asarray(k_norm_w, np.float32)

    # rope tables with norm weights folded in; sin table carries the
    # rotate-half sign on its first half and the rolled weights
    qwr, kwr = np.roll(qw, -64), np.roll(kw, -64)
    sgn = np.ones((1, D), np.float32)
    sgn[:, :64] = -1.0
    cosq, cosk = cos * qw, cos * kw
    sinq, sink_ = sin * qwr * sgn, sin * kwr * sgn
    cos3 = np.concatenate([cosq, cosq, cosk], 1).astype(BF)
    sin3 = np.concatenate([sinq, sinq, sink_], 1).astype(BF)

    kk = np.arange(128)[:, None]
    qq = np.arange(128)[None, :]
    mdiag = (kk <= qq).astype(np.float32)
    medge = (kk >= qq).astype(np.float32)
    mask = np.concatenate([mdiag, medge], 1).astype(BF)
    ident = np.eye(128, dtype=np.float32).astype(BF)
    esink_full = np.exp(sinks).astype(np.float32)

    in_maps = []
    for c in range(NCORES):
        kvh = c // 2
        wqkv = np.concatenate([wq[2 * c * 128:(2 * c + 2) * 128, :],
                               wk[kvh * 128:(kvh + 1) * 128, :],
                               wv[kvh * 128:(kvh + 1) * 128, :]], axis=0)
        wqkvTh = np.ascontiguousarray(wqkv.T).astype(BF)
        woTh = np.ascontiguousarray(wo[:, c * 256:(c + 1) * 256].T).astype(BF)
        in_maps.append(dict(
            xTb=xTh, wqkvT=wqkvTh, woTd=woTh, cosd=cos3, sind=sin3,
            maskd=mask, identd=ident,
            esinkd=np.ascontiguousarray(
                esink_full[2 * c:2 * c + 2].reshape(1, 2))))

    nc = _get_nc()
    res = run_bass_kernel_spmd(nc, in_maps, core_ids=list(range(NCORES)))
    total = np.zeros((HID, S), np.float32)
    for c in range(NCORES):
        total += np.asarray(res.results[c]["outT"], np.float32)
    return np.ascontiguousarray(total.T).reshape(1, S, HID)
